# revision 39
# baseline (speedup 1.0000x reference)
"""Trainium2 Bass kernel for the MoE block (nn_MoEBlock_5592047420171).

Strategy: data-parallel over tokens across 8 NeuronCores (1024 tokens/core,
all weights replicated; no collectives).  Per core, layout A (d_ff on
partitions, tokens on the free dim):

  out[t,:] = v_t * (relu(base_t + delta_{e1(t),t}) + relu(base_t + delta_{e2(t),t}))
             @ wo^T + 2 v_t * bo
  base = hs wi^T + bi,  delta_e = (hs lA_e^T) lB_e^T,  v = top2 softmax mass

Key structure (v2):
  * everything bf16 on the PE (fp8 measured numerically at 3.5-6% rel err
    on this problem -- over the 2e-2 gate -- so no DoubleRow).
  * router runs in bf16 straight from the shared hsB activation layout
    (simulated end-to-end err 0.39%, top-2 flips 0.07%); gate bias rides a
    DVE tensor_scalar on the logit PSUM.  No fp32 hs layout at all.
  * per-tile PE work is 10 matmuls: 8 base chunks + lora-B d1 folded into
    the same PSUM accumulation group, relu#1 on ACT reads the bank, then
    the PE adds lB*(tA2-tA1) into the SAME bank (+=) and relu#2 runs on
    DVE.  No identity matmul, no separate delta banks.
  * choice masks M1 / (M2-M1) are built with two tiny PE gather-matmuls
    from the one-hot transpose (no DRAM mask round-trips).
  * val_sum and bo are pulled out of the ff contraction entirely: H holds
    the unscaled relu sum; the output stage computes (psum + 2bo)*v with
    an ACT Copy + one DVE scalar_tensor_tensor while draining PSUM.
  * the first SPILL f-chunks run in a bank-spilling variant (base copied
    to SBUF via ACT Copy; bi folded into the later DVE combine) so the PE
    stays busy during the router pipeline without holding banks open;
    their PE-light completions are interleaved 1:2 among the full tiles
    so their DVE chains hide under full-tile matmuls.  NOTE: a 1:3
    interleave deterministically mis-schedules (rel err 2.4e-2) -- keep
    the 1:2 pattern.
  * DMA issue is split between the sync queue (router-critical path +
    wo stream) and the gpsimd queue (hs half-1 / wi / lb streams), hs is
    split into per-half tiles to keep DMA deps fine-grained, wi/lb are
    prefetched 3 f-chunks ahead, and a ~3us block of identity matmuls
    ramps the PE clock before the DMA-paced router; ACT sticks to
    Copy/Exp/Relu to avoid activation-table reloads.  Output is bf16
    (host upcasts) to halve the final output flush.
"""

import numpy as np
from contextlib import ExitStack

import concourse.bass as bass
import concourse.tile as tile
from concourse import bacc, mybir
from concourse.bass_utils import run_bass_kernel_spmd
from concourse.masks import make_identity

B, S, DM, FF, E, RK = 4, 2048, 1024, 4096, 8, 16
NCORES = 8
TOK = B * S            # 8192 tokens
T = TOK // NCORES      # 1024 tokens per core
TCH = T // 128         # 8 token chunks of 128
FCH = FF // 128        # 32 d_ff chunks of 128
TT = 512               # token tile width (free dim of big matmuls)
NTT = T // TT          # 2 token tiles
DCH = 8                # d_model chunks
SPILL = 4              # f-chunks run in the spill variant during the router

F32 = mybir.dt.float32
BF16 = mybir.dt.bfloat16
AX = mybir.AxisListType
ALU = mybir.AluOpType
AF = mybir.ActivationFunctionType


def build_bass():
    nc = bacc.Bacc("TRN2", target_bir_lowering=False)

    hsB = nc.declare_dram_parameter("hsB", [128, DCH, T], BF16, isOutput=False)
    wiB = nc.declare_dram_parameter("wiB", [FCH, 128, DCH, 128], BF16, isOutput=False)
    biC = nc.declare_dram_parameter("biC", [128, FCH], F32, isOutput=False)
    lAc = nc.declare_dram_parameter("lAc", [128, DCH, 128], BF16, isOutput=False)
    lBc = nc.declare_dram_parameter("lBc", [FCH, 128, 128], BF16, isOutput=False)
    gwB = nc.declare_dram_parameter("gwB", [128, DCH, 8], BF16, isOutput=False)
    gbC = nc.declare_dram_parameter("gbC", [8, 1], F32, isOutput=False)
    G1 = nc.declare_dram_parameter("G1", [8, 128], BF16, isOutput=False)
    Gpm = nc.declare_dram_parameter("Gpm", [16, 128], BF16, isOutput=False)
    woB = nc.declare_dram_parameter("woB", [FCH, 128, DM], BF16, isOutput=False)
    boC = nc.declare_dram_parameter("boC", [128, DCH], F32, isOutput=False)
    outT = nc.declare_dram_parameter("outT", [DM, T], BF16, isOutput=True)

    hsB, wiB, biC, lAc, lBc, gwB, gbC, G1, Gpm, woB, boC, outT = (
        h.ap() for h in (hsB, wiB, biC, lAc, lBc, gwB, gbC, G1, Gpm, woB,
                         boC, outT))

    with tile.TileContext(nc) as tc, ExitStack() as ctx:
        persist = ctx.enter_context(tc.tile_pool(name="persist", bufs=1))
        dram = ctx.enter_context(tc.tile_pool(name="dram", bufs=1, space="DRAM"))

        # identity built first: its gpsimd ops must not queue behind DMAs
        ident = persist.tile([128, 128], F32, tag="ident")
        make_identity(nc, ident)
        # PE warm-up: ~3us of dependency-free matmuls ramp the tensor engine
        # to full clock before the DMA-paced router matmuls begin
        with tc.tile_pool(name="warm_ps", bufs=1, space="PSUM") as warm_ps:
            wps = warm_ps.tile([128, 128], F32, tag="wps")
            for _ in range(15):
                nc.tensor.matmul(wps, lhsT=ident, rhs=ident,
                                 start=True, stop=True)

        # ---- resident tensors; router-critical DMAs go first on sync ----
        # hs is split into two per-half tiles so half-0 matmuls only wait on
        # the sync/scalar-queued half-0 DMAs (DMA deps are tile-granular).
        gw_sb = persist.tile([128, DCH, 8], BF16, tag="gw")
        nc.sync.dma_start(out=gw_sb, in_=gwB)
        hs_sb = [persist.tile([128, DCH, TT], BF16, tag=f"hs{h}",
                              name=f"hs{h}")
                 for h in range(NTT)]
        for ci in range(DCH):   # half 0 alternates sync / scalar queues
            eng = nc.sync if ci % 2 == 0 else nc.scalar
            eng.dma_start(out=hs_sb[0][:, ci, :], in_=hsB[:, ci, 0:TT])
        gb_sb = persist.tile([8, 1], F32, tag="gb")
        nc.scalar.dma_start(out=gb_sb, in_=gbC)
        # gpsimd order: first two spill-weight sets (needed ~t=12us), then
        # hs half 1, then the small constants, then the rest of the spills
        lA_sb = persist.tile([128, DCH, 128], BF16, tag="lA")
        bi_sb = persist.tile([128, FCH], F32, tag="bi")
        G1_sb = persist.tile([8, 128], BF16, tag="G1")
        Gpm_sb = persist.tile([16, 128], BF16, tag="Gpm")
        bo_sb = persist.tile([128, DCH], F32, tag="bo")

        wo_all = persist.tile([128, FCH, DM], BF16, tag="woall")

        ohT_sb = persist.tile([16, T], BF16, tag="ohT")   # oh1 rows 0-7, oh2 8-15
        vT_sb = persist.tile([1, T], F32, tag="vT")       # val_sum row
        V_b = persist.tile([128, T], F32, tag="Vb")       # val_sum bcast (f32)
        M1_sb = persist.tile([128, T], BF16, tag="M1")    # first-choice mask
        Md_sb = persist.tile([128, T], BF16, tag="Md")    # (M2 - M1) mask
        tA1_sb = persist.tile([128, T], BF16, tag="tA1")  # masked lora-A (1st)
        tAd_sb = persist.tile([128, T], BF16, tag="tAd")  # masked lora-A (2nd-1st)
        H_sb = [persist.tile([128, T], BF16, tag=f"H{fc}", name=f"H{fc}")
                for fc in range(FCH)]
        vrow = dram.tile([1, T], F32, tag="vrow")

        # pools that live through router + phase 3 (closed before phase 4)
        p3 = ExitStack()
        wi_pool = p3.enter_context(tc.tile_pool(name="wi_sb", bufs=6))
        lb_pool = p3.enter_context(tc.tile_pool(name="lb_sb", bufs=12))
        bank_pool = p3.enter_context(
            tc.tile_pool(name="bank_ps", bufs=4, space="PSUM"))
        bs_pool = p3.enter_context(
            tc.tile_pool(name="bs_sb", bufs=2 * SPILL))
        r1_pool = p3.enter_context(tc.tile_pool(name="r1_sb", bufs=5))
        r2_pool = p3.enter_context(tc.tile_pool(name="r2_sb", bufs=5))

        def fetch_w(fc):
            wi_cur = wi_pool.tile([128, DCH, 128], BF16, tag="wi",
                                  name=f"wi{fc}")
            nc.gpsimd.dma_start(out=wi_cur, in_=wiB[fc])
            lb_cur = lb_pool.tile([128, 128], BF16, tag="lb", name=f"lb{fc}")
            nc.gpsimd.dma_start(out=lb_cur, in_=lBc[fc])
            return wi_cur, lb_cur

        def fetch_wo(fc):
            nc.sync.dma_start(out=wo_all[:, fc, :], in_=woB[fc])

        def emit_base(fc, tt, wi_cur, close):
            tsl = slice(tt * TT, (tt + 1) * TT)
            bank = bank_pool.tile([128, TT], F32, tag="bank",
                                  name=f"bank{fc}_{tt}")
            for ci in range(DCH):
                nc.tensor.matmul(
                    bank,
                    lhsT=wi_cur[:, ci, :],
                    rhs=hs_sb[tt][:, ci, :],
                    start=(ci == 0), stop=(close and ci == DCH - 1),
                )
            return bank

        # prefetch spill-tile weights on gpsimd before the wi/wo stream
        warm_w = {fc: fetch_w(fc) for fc in range(2)}
        for ci in range(DCH):   # half 1 follows the first spill weights
            nc.gpsimd.dma_start(out=hs_sb[1][:, ci, :], in_=hsB[:, ci, TT:T])
        nc.gpsimd.dma_start(out=lA_sb, in_=lAc)
        nc.gpsimd.dma_start(out=bi_sb, in_=biC)
        nc.gpsimd.dma_start(out=G1_sb, in_=G1)
        nc.gpsimd.dma_start(out=Gpm_sb, in_=Gpm)
        nc.gpsimd.dma_start(out=bo_sb, in_=boC)
        for fc in range(2, SPILL):
            warm_w[fc] = fetch_w(fc)

        spill_bs = {}
        spill_list = [(fc, tt) for fc in range(SPILL) for tt in range(NTT)]

        def emit_spill(fc, tt):
            bank = emit_base(fc, tt, warm_w[fc][0], close=True)
            bs = bs_pool.tile([128, TT], BF16, tag="bs", name=f"bs{fc}_{tt}")
            nc.scalar.copy(out=bs, in_=bank)
            spill_bs[(fc, tt)] = bs

        # ---- phase 1: router ----
        with (
            tc.tile_pool(name="lg_ps", bufs=2, space="PSUM") as lg_ps,
            tc.tile_pool(name="rtr_ps", bufs=2, space="PSUM") as rtr_ps,
            tc.tile_pool(name="r_sb", bufs=2) as r_sb,
        ):
            LT_sb = r_sb.tile([8, T], F32, tag="LT", bufs=1)
            for h in range(NTT):
                hsl = slice(h * TT, (h + 1) * TT)
                lgT = lg_ps.tile([8, TT], F32, tag="lgT")
                for ci in range(DCH):
                    nc.tensor.matmul(
                        lgT,
                        lhsT=gw_sb[:, ci, :],
                        rhs=hs_sb[h][:, ci, :],
                        start=(ci == 0), stop=(ci == DCH - 1),
                    )
                nc.vector.tensor_scalar(
                    out=LT_sb[:, hsl], in0=lgT, scalar1=gb_sb,
                    scalar2=None, op0=ALU.add)
            si = 0
            for tch in range(TCH):
                tsl = slice(tch * 128, (tch + 1) * 128)
                rt0 = rtr_ps.tile([128, 128], F32, tag="rt")
                lg = rt0[:, 0:8]
                nc.tensor.transpose(lg, LT_sb[:, tsl], ident[0:8, 0:8])
                P = r_sb.tile([128, 8], F32, tag="P")
                nc.scalar.activation(P, lg, AF.Exp)
                top8 = r_sb.tile([128, 8], F32, tag="top8")
                nc.vector.max(out=top8, in_=P)
                sP = r_sb.tile([128, 1], F32, tag="sP")
                nc.vector.tensor_reduce(out=sP, in_=P, axis=AX.X, op=ALU.add)
                rv = r_sb.tile([128, 1], F32, tag="rv")
                nc.vector.reciprocal(rv, sP)
                ohb = r_sb.tile([128, 16], F32, tag="ohb")
                nc.vector.tensor_scalar(
                    out=ohb[:, 0:8], in0=P, scalar1=top8[:, 0:1], scalar2=None,
                    op0=ALU.is_equal,
                )
                nc.vector.scalar_tensor_tensor(
                    out=ohb[:, 8:16], in0=P, scalar=top8[:, 1:2],
                    in1=ohb[:, 0:8], op0=ALU.is_ge, op1=ALU.subtract,
                )
                vc = r_sb.tile([128, 1], F32, tag="vc")
                nc.vector.scalar_tensor_tensor(
                    out=vc, in0=top8[:, 0:1], scalar=top8[:, 1:2],
                    in1=rv, op0=ALU.add, op1=ALU.mult,
                )
                rt1 = rtr_ps.tile([128, 128], F32, tag="rt")
                trp = rt1[0:16, :]
                nc.tensor.transpose(trp, ohb, ident)
                rt2 = rtr_ps.tile([128, 128], F32, tag="rt")
                trpv = rt2[0:1, :]
                nc.tensor.transpose(trpv, vc, ident)
                nc.scalar.copy(out=ohT_sb[:, tsl], in_=trp)
                nc.vector.tensor_scalar(
                    out=vT_sb[:, tsl], in0=trpv, scalar1=1.0, scalar2=None,
                    op0=ALU.mult)
                # keep the PE fed while the ACT/DVE router chain runs
                if si < len(spill_list):
                    emit_spill(*spill_list[si]); si += 1
            while si < len(spill_list):
                emit_spill(*spill_list[si]); si += 1

        nc.sync.dma_start(out=vrow, in_=vT_sb)
        nc.sync.dma_start(out=V_b, in_=vrow.to_broadcast([128, T]))

        # ---- masks via PE gather-matmuls; lora-A projections ----
        with tc.tile_pool(name="seq_ps", bufs=3, space="PSUM") as seq_ps:
            for h in range(NTT):
                hsl = slice(h * TT, (h + 1) * TT)
                mp = seq_ps.tile([128, TT], F32, tag="sq")
                nc.tensor.matmul(mp, lhsT=G1_sb, rhs=ohT_sb[0:8, hsl],
                                 start=True, stop=True)
                nc.scalar.copy(out=M1_sb[:, hsl], in_=mp)
                md = seq_ps.tile([128, TT], F32, tag="sq")
                nc.tensor.matmul(md, lhsT=Gpm_sb, rhs=ohT_sb[:, hsl],
                                 start=True, stop=True)
                nc.scalar.copy(out=Md_sb[:, hsl], in_=md)
            for tt in range(NTT):
                tsl = slice(tt * TT, (tt + 1) * TT)
                pta = seq_ps.tile([128, TT], F32, tag="sq")
                for ci in range(DCH):
                    nc.tensor.matmul(
                        pta,
                        lhsT=lA_sb[:, ci, :],
                        rhs=hs_sb[tt][:, ci, :],
                        start=(ci == 0), stop=(ci == DCH - 1),
                    )
                nc.vector.scalar_tensor_tensor(
                    out=tA1_sb[:, tsl], in0=pta, scalar=1.0,
                    in1=M1_sb[:, tsl], op0=ALU.mult, op1=ALU.mult)
                nc.vector.scalar_tensor_tensor(
                    out=tAd_sb[:, tsl], in0=pta, scalar=1.0,
                    in1=Md_sb[:, tsl], op0=ALU.mult, op1=ALU.mult)

        # ---- phase 3: main loop over (f-chunk, t-tile), software-pipelined --
        # spill completions (PE-light, DVE-heavy) are interleaved among the
        # first full tiles so their DVE chains hide under full-tile matmuls.
        # their d-banks come from a second PSUM pool so their (slow) DVE
        # drains never starve the full-tile bank rotation.
        bank2_pool = p3.enter_context(
            tc.tile_pool(name="bank2_ps", bufs=4, space="PSUM"))
        spill_tiles = [(fc, tt) for fc in range(SPILL) for tt in range(NTT)]
        full_tiles = [(fc, tt) for fc in range(SPILL, FCH) for tt in range(NTT)]
        tiles = []
        for k, ft in enumerate(full_tiles):
            tiles.append(ft)
            if k % 2 == 1 and spill_tiles:
                tiles.append(spill_tiles.pop(0))
        tiles.extend(spill_tiles)
        pend = {}

        def flush(j, act_r2=False):
            kind, fcj, tslj, bankj, lbj, aux = pend.pop(j)
            nc.tensor.matmul(
                bankj, lhsT=lbj, rhs=tAd_sb[:, tslj],
                start=False, stop=True,
            )
            if kind == "full" and act_r2:
                r1j = aux
                r2 = r2_pool.tile([128, TT], BF16, tag="r2")
                nc.scalar.activation(r2, bankj, AF.Relu,
                                     bias=bi_sb[:, fcj:fcj + 1])
                nc.vector.tensor_tensor(
                    out=H_sb[fcj][:, tslj], in0=r1j, in1=r2, op=ALU.add)
                return
            if kind == "spill":
                bsj, r1j = aux
                x2 = r2_pool.tile([128, TT], BF16, tag="r2")
                nc.vector.scalar_tensor_tensor(
                    out=x2, in0=bankj, scalar=bi_sb[:, fcj:fcj + 1], in1=bsj,
                    op0=ALU.add, op1=ALU.add)
                nc.vector.scalar_tensor_tensor(
                    out=H_sb[fcj][:, tslj], in0=x2, scalar=0.0,
                    in1=r1j, op0=ALU.max, op1=ALU.add)
            else:
                r1j = aux
                r2 = r2_pool.tile([128, TT], BF16, tag="r2")
                nc.vector.tensor_scalar(
                    out=r2, in0=bankj, scalar1=bi_sb[:, fcj:fcj + 1],
                    scalar2=0.0, op0=ALU.add, op1=ALU.max)
                nc.vector.tensor_tensor(
                    out=H_sb[fcj][:, tslj], in0=r1j, in1=r2, op=ALU.add)

        # prefetch wi/lb three f-chunks ahead so base matmuls never wait
        wseq = [fc for fc, tt in tiles if fc >= SPILL and tt == 0]
        wmap = {}
        for fc in wseq[:5]:
            wmap[fc] = fetch_w(fc)
        wnext = 5
        wi_cur = lb_cur = None
        for i, (fc, tt) in enumerate(tiles):
            tsl = slice(tt * TT, (tt + 1) * TT)
            if fc < SPILL:
                lb_cur = warm_w[fc][1]
                bank = bank2_pool.tile([128, TT], F32, tag="bank",
                                       name=f"dbank{fc}_{tt}")
                nc.tensor.matmul(bank, lhsT=lb_cur, rhs=tA1_sb[:, tsl],
                                 start=True, stop=False)
                bs = spill_bs[(fc, tt)]
                x1 = r1_pool.tile([128, TT], BF16, tag="r1")
                nc.vector.scalar_tensor_tensor(
                    out=x1, in0=bank, scalar=bi_sb[:, fc:fc + 1], in1=bs,
                    op0=ALU.add, op1=ALU.add)
                r1 = r1_pool.tile([128, TT], BF16, tag="r1")
                nc.scalar.activation(r1, x1, AF.Relu)
                entry = ("spill", fc, tsl, bank, lb_cur, (bs, r1))
            else:
                if tt == 0:
                    wi_cur, lb_cur = wmap.pop(fc)
                    if wnext < len(wseq):
                        wmap[wseq[wnext]] = fetch_w(wseq[wnext])
                        wnext += 1
                bank = emit_base(fc, tt, wi_cur, close=False)
                nc.tensor.matmul(
                    bank, lhsT=lb_cur, rhs=tA1_sb[:, tsl],
                    start=False, stop=False,
                )
                r1 = r1_pool.tile([128, TT], BF16, tag="r1")
                nc.scalar.activation(r1, bank, AF.Relu,
                                     bias=bi_sb[:, fc:fc + 1])
                entry = ("full", fc, tsl, bank, lb_cur, r1)
            if i < FCH:
                fetch_wo(i)          # stream the 8MB wo copy behind the loop
            if i - 1 in pend:
                flush(i - 1, act_r2=(i >= len(tiles) - 2))
            pend[i] = entry
        flush(len(tiles) - 1, act_r2=True)

        p3.close()  # free phase-3 pools (incl. PSUM banks) before phase 4

        # ---- phase 4: wo matmul; output = (psum + 2 bo) * v ----
        with (
            tc.tile_pool(name="wo_ps", bufs=1, space="PSUM") as wo_ps,
            tc.tile_pool(name="o_sb", bufs=6) as o_pool,
        ):
            for tt in range(NTT):
                tsl = slice(tt * TT, (tt + 1) * TT)
                ops = [wo_ps.tile([128, TT], F32, tag=f"o{dc}",
                                  name=f"o{tt}_{dc}")
                       for dc in range(DCH)]
                for fc in range(FCH):
                    for dc in range(DCH):
                        nc.tensor.matmul(
                            ops[dc],
                            lhsT=wo_all[:, fc, dc * 128:(dc + 1) * 128],
                            rhs=H_sb[fc][:, tsl],
                            start=(fc == 0), stop=(fc == FCH - 1),
                        )
                for dc in range(DCH):
                    o_t = o_pool.tile([128, TT], BF16, tag="ot")
                    if dc % 2 == 0:
                        nc.vector.scalar_tensor_tensor(
                            out=o_t, in0=ops[dc], scalar=bo_sb[:, dc:dc + 1],
                            in1=V_b[:, tsl], op0=ALU.add, op1=ALU.mult)
                    else:
                        t_o = o_pool.tile([128, TT], F32, tag="to")
                        nc.scalar.copy(out=t_o, in_=ops[dc])
                        nc.vector.scalar_tensor_tensor(
                            out=o_t, in0=t_o, scalar=bo_sb[:, dc:dc + 1],
                            in1=V_b[:, tsl], op0=ALU.add, op1=ALU.mult)
                    eng = nc.sync if dc % 2 == 0 else nc.scalar
                    eng.dma_start(
                        out=outT[dc * 128:(dc + 1) * 128, tsl], in_=o_t
                    )

    nc.compile()
    return nc


def prep_inputs(hidden_states, wi, bi, wo, bo, lora_A, lora_B, gate_w, gate_b):
    """Host-side layout prep; returns per-core input maps."""
    import ml_dtypes
    bf = ml_dtypes.bfloat16
    f32 = np.float32
    hs = np.asarray(hidden_states, f32).reshape(TOK, DM)
    wi = np.asarray(wi, f32); bi = np.asarray(bi, f32)
    wo = np.asarray(wo, f32); bo = np.asarray(bo, f32)
    lora_A = np.asarray(lora_A, f32); lora_B = np.asarray(lora_B, f32)
    gate_w = np.asarray(gate_w, f32); gate_b = np.asarray(gate_b, f32)

    # wi^T in (fc, d, ci, f) bf16 chunks
    wiB = np.ascontiguousarray(
        wi.T.reshape(DCH, 128, FCH, 128).transpose(2, 1, 0, 3)).astype(bf)
    biC = np.ascontiguousarray(bi.reshape(FCH, 128).T)

    # lora-A concatenated: columns 16e+r = lora_A[e,r,:]
    lA_cat = np.concatenate([lora_A[e].T for e in range(E)], axis=1)  # [DM,128]
    lAc = np.ascontiguousarray(lA_cat.reshape(DCH, 128, 128).transpose(1, 0, 2)
                               ).astype(bf)
    # lora-B concatenated: rows 16e+r = lora_B[e,:,r]
    lB_cat = np.concatenate([lora_B[e].T for e in range(E)], axis=0)  # [128,FF]
    lBc = np.ascontiguousarray(
        lB_cat.reshape(128, FCH, 128).transpose(1, 0, 2)).astype(bf)

    gwB = np.ascontiguousarray(
        gate_w.T.reshape(DCH, 128, E).transpose(1, 0, 2)).astype(bf)
    gbC = gate_b.reshape(E, 1).astype(f32)

    G1 = np.zeros((E, 128), f32)
    for e in range(E):
        G1[e, 16 * e:16 * (e + 1)] = 1.0
    Gpm = np.concatenate([-G1, G1], axis=0)  # M2 - M1 gather

    woB = np.ascontiguousarray(wo.T.reshape(FCH, 128, DM)).astype(bf)
    boC = np.ascontiguousarray((2.0 * bo).reshape(DCH, 128).T).astype(f32)

    shared = dict(wiB=wiB, biC=biC, lAc=lAc, lBc=lBc, gwB=gwB, gbC=gbC,
                  G1=G1.astype(bf), Gpm=Gpm.astype(bf), woB=woB, boC=boC)
    in_maps = []
    for c in range(NCORES):
        hsc = hs[c * T:(c + 1) * T]
        hsB = np.ascontiguousarray(
            hsc.T.reshape(DCH, 128, T).transpose(1, 0, 2)).astype(bf)
        in_maps.append(dict(hsB=hsB, **shared))
    return in_maps


def run(in_maps, **kwargs):
    nc = build_bass()
    return nc, run_bass_kernel_spmd(nc, in_maps, list(range(NCORES)), **kwargs)


def kernel(hidden_states, wi, bi, wo, bo, lora_A, lora_B, gate_w, gate_b):
    in_maps = prep_inputs(hidden_states, wi, bi, wo, bo, lora_A, lora_B,
                          gate_w, gate_b)
    _, res = run(in_maps)
    out = np.stack([res.results[c]["outT"].T for c in range(NCORES)])
    return out.reshape(B, S, DM).astype(np.float32)


# revision 40
# speedup vs baseline: 1.0124x; 1.0124x over previous
"""Trainium2 Bass kernel for the MoE block (nn_MoEBlock_5592047420171).

Strategy: data-parallel over tokens across 8 NeuronCores (1024 tokens/core,
all weights replicated; no collectives).  Per core, layout A (d_ff on
partitions, tokens on the free dim):

  out[t,:] = v_t * (relu(base_t + delta_{e1(t),t}) + relu(base_t + delta_{e2(t),t}))
             @ wo^T + 2 v_t * bo
  base = hs wi^T + bi,  delta_e = (hs lA_e^T) lB_e^T,  v = top2 softmax mass

Key structure (v2):
  * everything bf16 on the PE (fp8 measured numerically at 3.5-6% rel err
    on this problem -- over the 2e-2 gate -- so no DoubleRow).
  * router runs in bf16 straight from the shared hsB activation layout
    (simulated end-to-end err 0.39%, top-2 flips 0.07%); gate bias rides a
    DVE tensor_scalar on the logit PSUM.  No fp32 hs layout at all.
  * per-tile PE work is 10 matmuls: 8 base chunks + lora-B d1 folded into
    the same PSUM accumulation group, relu#1 on ACT reads the bank, then
    the PE adds lB*(tA2-tA1) into the SAME bank (+=) and relu#2 runs on
    DVE.  No identity matmul, no separate delta banks.
  * choice masks M1 / (M2-M1) are built with two tiny PE gather-matmuls
    from the one-hot transpose (no DRAM mask round-trips).
  * val_sum and bo are pulled out of the ff contraction entirely: H holds
    the unscaled relu sum; the output stage computes (psum + 2bo)*v with
    an ACT Copy + one DVE scalar_tensor_tensor while draining PSUM.
  * the first SPILL f-chunks run in a bank-spilling variant (base copied
    to SBUF via ACT Copy; bi folded into the later DVE combine) so the PE
    stays busy during the router pipeline without holding banks open;
    their PE-light completions are interleaved 1:2 among the full tiles
    so their DVE chains hide under full-tile matmuls.  NOTE: a 1:3
    interleave deterministically mis-schedules (rel err 2.4e-2) -- keep
    the 1:2 pattern.
  * DMA issue is split between the sync queue (router-critical path +
    wo stream) and the gpsimd queue (hs half-1 / wi / lb streams), hs is
    split into per-half tiles to keep DMA deps fine-grained, wi/lb are
    prefetched 3 f-chunks ahead, and a ~3us block of identity matmuls
    ramps the PE clock before the DMA-paced router; ACT sticks to
    Copy/Exp/Relu to avoid activation-table reloads.  Output is bf16
    (host upcasts) to halve the final output flush.
"""

import numpy as np
from contextlib import ExitStack

import concourse.bass as bass
import concourse.tile as tile
from concourse import bacc, mybir
from concourse.bass_utils import run_bass_kernel_spmd
from concourse.masks import make_identity

B, S, DM, FF, E, RK = 4, 2048, 1024, 4096, 8, 16
NCORES = 8
TOK = B * S            # 8192 tokens
T = TOK // NCORES      # 1024 tokens per core
TCH = T // 128         # 8 token chunks of 128
FCH = FF // 128        # 32 d_ff chunks of 128
TT = 512               # token tile width (free dim of big matmuls)
NTT = T // TT          # 2 token tiles
DCH = 8                # d_model chunks
SPILL = 5              # f-chunks run in the spill variant during the router

F32 = mybir.dt.float32
BF16 = mybir.dt.bfloat16
AX = mybir.AxisListType
ALU = mybir.AluOpType
AF = mybir.ActivationFunctionType


def build_bass():
    nc = bacc.Bacc("TRN2", target_bir_lowering=False)

    hsB = nc.declare_dram_parameter("hsB", [128, DCH, T], BF16, isOutput=False)
    wiB = nc.declare_dram_parameter("wiB", [FCH, 128, DCH, 128], BF16, isOutput=False)
    biC = nc.declare_dram_parameter("biC", [128, FCH], F32, isOutput=False)
    lAc = nc.declare_dram_parameter("lAc", [128, DCH, 128], BF16, isOutput=False)
    lBc = nc.declare_dram_parameter("lBc", [FCH, 128, 128], BF16, isOutput=False)
    gwB = nc.declare_dram_parameter("gwB", [128, DCH, 8], BF16, isOutput=False)
    gbC = nc.declare_dram_parameter("gbC", [8, 1], F32, isOutput=False)
    G1 = nc.declare_dram_parameter("G1", [8, 128], BF16, isOutput=False)
    Gpm = nc.declare_dram_parameter("Gpm", [16, 128], BF16, isOutput=False)
    woB = nc.declare_dram_parameter("woB", [FCH, 128, DM], BF16, isOutput=False)
    boC = nc.declare_dram_parameter("boC", [128, DCH], F32, isOutput=False)
    outT = nc.declare_dram_parameter("outT", [DM, T], BF16, isOutput=True)

    hsB, wiB, biC, lAc, lBc, gwB, gbC, G1, Gpm, woB, boC, outT = (
        h.ap() for h in (hsB, wiB, biC, lAc, lBc, gwB, gbC, G1, Gpm, woB,
                         boC, outT))

    with tile.TileContext(nc) as tc, ExitStack() as ctx:
        persist = ctx.enter_context(tc.tile_pool(name="persist", bufs=1))
        dram = ctx.enter_context(tc.tile_pool(name="dram", bufs=1, space="DRAM"))

        # identity built first: its gpsimd ops must not queue behind DMAs
        ident = persist.tile([128, 128], F32, tag="ident")
        make_identity(nc, ident)
        # PE warm-up: ~3us of dependency-free matmuls ramp the tensor engine
        # to full clock before the DMA-paced router matmuls begin
        with tc.tile_pool(name="warm_ps", bufs=1, space="PSUM") as warm_ps:
            wps = warm_ps.tile([128, 128], F32, tag="wps")
            for _ in range(11):
                nc.tensor.matmul(wps, lhsT=ident, rhs=ident,
                                 start=True, stop=True)

        # ---- resident tensors; router-critical DMAs go first on sync ----
        # hs is split into two per-half tiles so half-0 matmuls only wait on
        # the sync/scalar-queued half-0 DMAs (DMA deps are tile-granular).
        gw_sb = persist.tile([128, DCH, 8], BF16, tag="gw")
        nc.sync.dma_start(out=gw_sb, in_=gwB)
        hs_sb = [persist.tile([128, DCH, TT], BF16, tag=f"hs{h}",
                              name=f"hs{h}")
                 for h in range(NTT)]
        for ci in range(DCH):   # half 0 alternates sync / scalar queues
            eng = nc.sync if ci % 2 == 0 else nc.scalar
            eng.dma_start(out=hs_sb[0][:, ci, :], in_=hsB[:, ci, 0:TT])
        gb_sb = persist.tile([8, 1], F32, tag="gb")
        nc.scalar.dma_start(out=gb_sb, in_=gbC)
        # gpsimd order: first two spill-weight sets (needed ~t=12us), then
        # hs half 1, then the small constants, then the rest of the spills
        lA_sb = persist.tile([128, DCH, 128], BF16, tag="lA")
        bi_sb = persist.tile([128, FCH], F32, tag="bi")
        G1_sb = persist.tile([8, 128], BF16, tag="G1")
        Gpm_sb = persist.tile([16, 128], BF16, tag="Gpm")
        bo_sb = persist.tile([128, DCH], F32, tag="bo")

        wo_all = persist.tile([128, FCH, DM], BF16, tag="woall")

        ohT_sb = persist.tile([16, T], BF16, tag="ohT")   # oh1 rows 0-7, oh2 8-15
        vT_sb = persist.tile([1, T], F32, tag="vT")       # val_sum row
        V_b = persist.tile([128, T], F32, tag="Vb")       # val_sum bcast (f32)
        M1_sb = persist.tile([128, T], BF16, tag="M1")    # first-choice mask
        Md_sb = persist.tile([128, T], BF16, tag="Md")    # (M2 - M1) mask
        tA1_sb = persist.tile([128, T], BF16, tag="tA1")  # masked lora-A (1st)
        tAd_sb = persist.tile([128, T], BF16, tag="tAd")  # masked lora-A (2nd-1st)
        H_sb = [persist.tile([128, T], BF16, tag=f"H{fc}", name=f"H{fc}")
                for fc in range(FCH)]
        vrow = dram.tile([1, T], F32, tag="vrow")

        # pools that live through router + phase 3 (closed before phase 4)
        p3 = ExitStack()
        wi_pool = p3.enter_context(tc.tile_pool(name="wi_sb", bufs=6))
        lb_pool = p3.enter_context(tc.tile_pool(name="lb_sb", bufs=12))
        bank_pool = p3.enter_context(
            tc.tile_pool(name="bank_ps", bufs=4, space="PSUM"))
        bs_pool = p3.enter_context(
            tc.tile_pool(name="bs_sb", bufs=2 * SPILL))
        r1_pool = p3.enter_context(tc.tile_pool(name="r1_sb", bufs=5))
        r2_pool = p3.enter_context(tc.tile_pool(name="r2_sb", bufs=5))

        def fetch_w(fc):
            wi_cur = wi_pool.tile([128, DCH, 128], BF16, tag="wi",
                                  name=f"wi{fc}")
            nc.gpsimd.dma_start(out=wi_cur, in_=wiB[fc])
            lb_cur = lb_pool.tile([128, 128], BF16, tag="lb", name=f"lb{fc}")
            nc.gpsimd.dma_start(out=lb_cur, in_=lBc[fc])
            return wi_cur, lb_cur

        def fetch_wo(fc):
            nc.sync.dma_start(out=wo_all[:, fc, :], in_=woB[fc])

        def emit_base(fc, tt, wi_cur, close):
            tsl = slice(tt * TT, (tt + 1) * TT)
            bank = bank_pool.tile([128, TT], F32, tag="bank",
                                  name=f"bank{fc}_{tt}")
            for ci in range(DCH):
                nc.tensor.matmul(
                    bank,
                    lhsT=wi_cur[:, ci, :],
                    rhs=hs_sb[tt][:, ci, :],
                    start=(ci == 0), stop=(close and ci == DCH - 1),
                )
            return bank

        # prefetch spill-tile weights on gpsimd before the wi/wo stream
        warm_w = {fc: fetch_w(fc) for fc in range(2)}
        for ci in range(DCH):   # half 1 follows the first spill weights
            nc.gpsimd.dma_start(out=hs_sb[1][:, ci, :], in_=hsB[:, ci, TT:T])
        nc.gpsimd.dma_start(out=lA_sb, in_=lAc)
        nc.gpsimd.dma_start(out=bi_sb, in_=biC)
        nc.gpsimd.dma_start(out=G1_sb, in_=G1)
        nc.gpsimd.dma_start(out=Gpm_sb, in_=Gpm)
        nc.gpsimd.dma_start(out=bo_sb, in_=boC)
        for fc in range(2, SPILL):
            warm_w[fc] = fetch_w(fc)

        spill_bs = {}
        spill_list = [(fc, tt) for fc in range(SPILL) for tt in range(NTT)]

        def emit_spill(fc, tt):
            bank = emit_base(fc, tt, warm_w[fc][0], close=True)
            bs = bs_pool.tile([128, TT], BF16, tag="bs", name=f"bs{fc}_{tt}")
            nc.scalar.copy(out=bs, in_=bank)
            spill_bs[(fc, tt)] = bs

        # ---- phase 1: router ----
        with (
            tc.tile_pool(name="lg_ps", bufs=2, space="PSUM") as lg_ps,
            tc.tile_pool(name="rtr_ps", bufs=2, space="PSUM") as rtr_ps,
            tc.tile_pool(name="r_sb", bufs=2) as r_sb,
        ):
            LT_sb = r_sb.tile([8, T], F32, tag="LT", bufs=1)
            for h in range(NTT):
                hsl = slice(h * TT, (h + 1) * TT)
                lgT = lg_ps.tile([8, TT], F32, tag="lgT")
                for ci in range(DCH):
                    nc.tensor.matmul(
                        lgT,
                        lhsT=gw_sb[:, ci, :],
                        rhs=hs_sb[h][:, ci, :],
                        start=(ci == 0), stop=(ci == DCH - 1),
                    )
                nc.vector.tensor_scalar(
                    out=LT_sb[:, hsl], in0=lgT, scalar1=gb_sb,
                    scalar2=None, op0=ALU.add)
            si = 0
            for tch in range(TCH):
                tsl = slice(tch * 128, (tch + 1) * 128)
                rt0 = rtr_ps.tile([128, 128], F32, tag="rt")
                lg = rt0[:, 0:8]
                nc.tensor.transpose(lg, LT_sb[:, tsl], ident[0:8, 0:8])
                P = r_sb.tile([128, 8], F32, tag="P")
                nc.scalar.activation(P, lg, AF.Exp)
                top8 = r_sb.tile([128, 8], F32, tag="top8")
                nc.vector.max(out=top8, in_=P)
                sP = r_sb.tile([128, 1], F32, tag="sP")
                nc.vector.tensor_reduce(out=sP, in_=P, axis=AX.X, op=ALU.add)
                rv = r_sb.tile([128, 1], F32, tag="rv")
                nc.vector.reciprocal(rv, sP)
                ohb = r_sb.tile([128, 16], F32, tag="ohb")
                nc.vector.tensor_scalar(
                    out=ohb[:, 0:8], in0=P, scalar1=top8[:, 0:1], scalar2=None,
                    op0=ALU.is_equal,
                )
                nc.vector.scalar_tensor_tensor(
                    out=ohb[:, 8:16], in0=P, scalar=top8[:, 1:2],
                    in1=ohb[:, 0:8], op0=ALU.is_ge, op1=ALU.subtract,
                )
                vc = r_sb.tile([128, 1], F32, tag="vc")
                nc.vector.scalar_tensor_tensor(
                    out=vc, in0=top8[:, 0:1], scalar=top8[:, 1:2],
                    in1=rv, op0=ALU.add, op1=ALU.mult,
                )
                rt1 = rtr_ps.tile([128, 128], F32, tag="rt")
                trp = rt1[0:16, :]
                nc.tensor.transpose(trp, ohb, ident)
                rt2 = rtr_ps.tile([128, 128], F32, tag="rt")
                trpv = rt2[0:1, :]
                nc.tensor.transpose(trpv, vc, ident)
                nc.scalar.copy(out=ohT_sb[:, tsl], in_=trp)
                nc.vector.tensor_scalar(
                    out=vT_sb[:, tsl], in0=trpv, scalar1=1.0, scalar2=None,
                    op0=ALU.mult)
                # keep the PE fed while the ACT/DVE router chain runs
                if si < len(spill_list):
                    emit_spill(*spill_list[si]); si += 1
            while si < len(spill_list):
                emit_spill(*spill_list[si]); si += 1

        nc.sync.dma_start(out=vrow, in_=vT_sb)
        nc.sync.dma_start(out=V_b, in_=vrow.to_broadcast([128, T]))

        # ---- masks via PE gather-matmuls; lora-A projections ----
        with tc.tile_pool(name="seq_ps", bufs=3, space="PSUM") as seq_ps:
            for h in range(NTT):
                hsl = slice(h * TT, (h + 1) * TT)
                mp = seq_ps.tile([128, TT], F32, tag="sq")
                nc.tensor.matmul(mp, lhsT=G1_sb, rhs=ohT_sb[0:8, hsl],
                                 start=True, stop=True)
                nc.scalar.copy(out=M1_sb[:, hsl], in_=mp)
                md = seq_ps.tile([128, TT], F32, tag="sq")
                nc.tensor.matmul(md, lhsT=Gpm_sb, rhs=ohT_sb[:, hsl],
                                 start=True, stop=True)
                nc.scalar.copy(out=Md_sb[:, hsl], in_=md)
            for tt in range(NTT):
                tsl = slice(tt * TT, (tt + 1) * TT)
                pta = seq_ps.tile([128, TT], F32, tag="sq")
                for ci in range(DCH):
                    nc.tensor.matmul(
                        pta,
                        lhsT=lA_sb[:, ci, :],
                        rhs=hs_sb[tt][:, ci, :],
                        start=(ci == 0), stop=(ci == DCH - 1),
                    )
                nc.vector.scalar_tensor_tensor(
                    out=tA1_sb[:, tsl], in0=pta, scalar=1.0,
                    in1=M1_sb[:, tsl], op0=ALU.mult, op1=ALU.mult)
                nc.vector.scalar_tensor_tensor(
                    out=tAd_sb[:, tsl], in0=pta, scalar=1.0,
                    in1=Md_sb[:, tsl], op0=ALU.mult, op1=ALU.mult)

        # ---- phase 3: main loop over (f-chunk, t-tile), software-pipelined --
        # spill completions (PE-light, DVE-heavy) are interleaved among the
        # first full tiles so their DVE chains hide under full-tile matmuls.
        # their d-banks come from a second PSUM pool so their (slow) DVE
        # drains never starve the full-tile bank rotation.
        bank2_pool = p3.enter_context(
            tc.tile_pool(name="bank2_ps", bufs=4, space="PSUM"))
        spill_tiles = [(fc, tt) for fc in range(SPILL) for tt in range(NTT)]
        full_tiles = [(fc, tt) for fc in range(SPILL, FCH) for tt in range(NTT)]
        tiles = []
        for k, ft in enumerate(full_tiles):
            tiles.append(ft)
            if k % 2 == 1 and spill_tiles:
                tiles.append(spill_tiles.pop(0))
        tiles.extend(spill_tiles)
        pend = {}

        def flush(j, act_r2=False):
            kind, fcj, tslj, bankj, lbj, aux = pend.pop(j)
            nc.tensor.matmul(
                bankj, lhsT=lbj, rhs=tAd_sb[:, tslj],
                start=False, stop=True,
            )
            if kind == "full" and act_r2:
                r1j = aux
                r2 = r2_pool.tile([128, TT], BF16, tag="r2")
                nc.scalar.activation(r2, bankj, AF.Relu,
                                     bias=bi_sb[:, fcj:fcj + 1])
                nc.vector.tensor_tensor(
                    out=H_sb[fcj][:, tslj], in0=r1j, in1=r2, op=ALU.add)
                return
            if kind == "spill":
                bsj, r1j = aux
                x2 = r2_pool.tile([128, TT], BF16, tag="r2")
                nc.vector.scalar_tensor_tensor(
                    out=x2, in0=bankj, scalar=bi_sb[:, fcj:fcj + 1], in1=bsj,
                    op0=ALU.add, op1=ALU.add)
                nc.vector.scalar_tensor_tensor(
                    out=H_sb[fcj][:, tslj], in0=x2, scalar=0.0,
                    in1=r1j, op0=ALU.max, op1=ALU.add)
            else:
                r1j = aux
                r2 = r2_pool.tile([128, TT], BF16, tag="r2")
                nc.vector.tensor_scalar(
                    out=r2, in0=bankj, scalar1=bi_sb[:, fcj:fcj + 1],
                    scalar2=0.0, op0=ALU.add, op1=ALU.max)
                nc.vector.tensor_tensor(
                    out=H_sb[fcj][:, tslj], in0=r1j, in1=r2, op=ALU.add)

        # prefetch wi/lb three f-chunks ahead so base matmuls never wait
        wseq = [fc for fc, tt in tiles if fc >= SPILL and tt == 0]
        wmap = {}
        for fc in wseq[:5]:
            wmap[fc] = fetch_w(fc)
        wnext = 5
        wi_cur = lb_cur = None
        for i, (fc, tt) in enumerate(tiles):
            tsl = slice(tt * TT, (tt + 1) * TT)
            if fc < SPILL:
                lb_cur = warm_w[fc][1]
                bank = bank2_pool.tile([128, TT], F32, tag="bank",
                                       name=f"dbank{fc}_{tt}")
                nc.tensor.matmul(bank, lhsT=lb_cur, rhs=tA1_sb[:, tsl],
                                 start=True, stop=False)
                bs = spill_bs[(fc, tt)]
                x1 = r1_pool.tile([128, TT], BF16, tag="r1")
                nc.vector.scalar_tensor_tensor(
                    out=x1, in0=bank, scalar=bi_sb[:, fc:fc + 1], in1=bs,
                    op0=ALU.add, op1=ALU.add)
                r1 = r1_pool.tile([128, TT], BF16, tag="r1")
                nc.scalar.activation(r1, x1, AF.Relu)
                entry = ("spill", fc, tsl, bank, lb_cur, (bs, r1))
            else:
                if tt == 0:
                    wi_cur, lb_cur = wmap.pop(fc)
                    if wnext < len(wseq):
                        wmap[wseq[wnext]] = fetch_w(wseq[wnext])
                        wnext += 1
                bank = emit_base(fc, tt, wi_cur, close=False)
                nc.tensor.matmul(
                    bank, lhsT=lb_cur, rhs=tA1_sb[:, tsl],
                    start=False, stop=False,
                )
                r1 = r1_pool.tile([128, TT], BF16, tag="r1")
                nc.scalar.activation(r1, bank, AF.Relu,
                                     bias=bi_sb[:, fc:fc + 1])
                entry = ("full", fc, tsl, bank, lb_cur, r1)
            if i < FCH:
                fetch_wo(i)          # stream the 8MB wo copy behind the loop
            if i - 1 in pend:
                flush(i - 1, act_r2=(i >= len(tiles) - 2))
            pend[i] = entry
        flush(len(tiles) - 1, act_r2=True)

        p3.close()  # free phase-3 pools (incl. PSUM banks) before phase 4

        # ---- phase 4: wo matmul; output = (psum + 2 bo) * v ----
        with (
            tc.tile_pool(name="wo_ps", bufs=1, space="PSUM") as wo_ps,
            tc.tile_pool(name="o_sb", bufs=6) as o_pool,
        ):
            for tt in range(NTT):
                tsl = slice(tt * TT, (tt + 1) * TT)
                ops = [wo_ps.tile([128, TT], F32, tag=f"o{dc}",
                                  name=f"o{tt}_{dc}")
                       for dc in range(DCH)]
                for fc in range(FCH):
                    for dc in range(DCH):
                        nc.tensor.matmul(
                            ops[dc],
                            lhsT=wo_all[:, fc, dc * 128:(dc + 1) * 128],
                            rhs=H_sb[fc][:, tsl],
                            start=(fc == 0), stop=(fc == FCH - 1),
                        )
                for dc in range(DCH):
                    o_t = o_pool.tile([128, TT], BF16, tag="ot")
                    if dc % 2 == 0:
                        nc.vector.scalar_tensor_tensor(
                            out=o_t, in0=ops[dc], scalar=bo_sb[:, dc:dc + 1],
                            in1=V_b[:, tsl], op0=ALU.add, op1=ALU.mult)
                    else:
                        t_o = o_pool.tile([128, TT], F32, tag="to")
                        nc.scalar.copy(out=t_o, in_=ops[dc])
                        nc.vector.scalar_tensor_tensor(
                            out=o_t, in0=t_o, scalar=bo_sb[:, dc:dc + 1],
                            in1=V_b[:, tsl], op0=ALU.add, op1=ALU.mult)
                    eng = nc.sync if dc % 2 == 0 else nc.scalar
                    eng.dma_start(
                        out=outT[dc * 128:(dc + 1) * 128, tsl], in_=o_t
                    )

    nc.compile()
    return nc


def prep_inputs(hidden_states, wi, bi, wo, bo, lora_A, lora_B, gate_w, gate_b):
    """Host-side layout prep; returns per-core input maps."""
    import ml_dtypes
    bf = ml_dtypes.bfloat16
    f32 = np.float32
    hs = np.asarray(hidden_states, f32).reshape(TOK, DM)
    wi = np.asarray(wi, f32); bi = np.asarray(bi, f32)
    wo = np.asarray(wo, f32); bo = np.asarray(bo, f32)
    lora_A = np.asarray(lora_A, f32); lora_B = np.asarray(lora_B, f32)
    gate_w = np.asarray(gate_w, f32); gate_b = np.asarray(gate_b, f32)

    # wi^T in (fc, d, ci, f) bf16 chunks
    wiB = np.ascontiguousarray(
        wi.T.reshape(DCH, 128, FCH, 128).transpose(2, 1, 0, 3)).astype(bf)
    biC = np.ascontiguousarray(bi.reshape(FCH, 128).T)

    # lora-A concatenated: columns 16e+r = lora_A[e,r,:]
    lA_cat = np.concatenate([lora_A[e].T for e in range(E)], axis=1)  # [DM,128]
    lAc = np.ascontiguousarray(lA_cat.reshape(DCH, 128, 128).transpose(1, 0, 2)
                               ).astype(bf)
    # lora-B concatenated: rows 16e+r = lora_B[e,:,r]
    lB_cat = np.concatenate([lora_B[e].T for e in range(E)], axis=0)  # [128,FF]
    lBc = np.ascontiguousarray(
        lB_cat.reshape(128, FCH, 128).transpose(1, 0, 2)).astype(bf)

    gwB = np.ascontiguousarray(
        gate_w.T.reshape(DCH, 128, E).transpose(1, 0, 2)).astype(bf)
    gbC = gate_b.reshape(E, 1).astype(f32)

    G1 = np.zeros((E, 128), f32)
    for e in range(E):
        G1[e, 16 * e:16 * (e + 1)] = 1.0
    Gpm = np.concatenate([-G1, G1], axis=0)  # M2 - M1 gather

    woB = np.ascontiguousarray(wo.T.reshape(FCH, 128, DM)).astype(bf)
    boC = np.ascontiguousarray((2.0 * bo).reshape(DCH, 128).T).astype(f32)

    shared = dict(wiB=wiB, biC=biC, lAc=lAc, lBc=lBc, gwB=gwB, gbC=gbC,
                  G1=G1.astype(bf), Gpm=Gpm.astype(bf), woB=woB, boC=boC)
    in_maps = []
    for c in range(NCORES):
        hsc = hs[c * T:(c + 1) * T]
        hsB = np.ascontiguousarray(
            hsc.T.reshape(DCH, 128, T).transpose(1, 0, 2)).astype(bf)
        in_maps.append(dict(hsB=hsB, **shared))
    return in_maps


def run(in_maps, **kwargs):
    nc = build_bass()
    return nc, run_bass_kernel_spmd(nc, in_maps, list(range(NCORES)), **kwargs)


def kernel(hidden_states, wi, bi, wo, bo, lora_A, lora_B, gate_w, gate_b):
    in_maps = prep_inputs(hidden_states, wi, bi, wo, bo, lora_A, lora_B,
                          gate_w, gate_b)
    _, res = run(in_maps)
    out = np.stack([res.results[c]["outT"].T for c in range(NCORES)])
    return out.reshape(B, S, DM).astype(np.float32)
